# revision 9
# baseline (speedup 1.0000x reference)
"""Distributed mean-squared-distance kernel for Trainium2 (8 NeuronCores).

Computes  out[b] = mean_n ||x[b] - features[n]||^2  for x:[1024,128],
features:[100000,128].

Because the mean is linear, the full [B, N] distance matrix is never needed:

    out[b] = ||x_b||^2 + (1/N) * sum_n ||f_n||^2 - (2/N) * x_b . (sum_n f_n)

Each core streams a 1/8 shard of `features` once (memory-bound roofline:
~6.55 MB/core at ~350 GB/s).  Engine split, all overlapped with the DMA
stream:

  * PE (tensor engine): S1 = sum_n f_n via an all-ones stationary matrix --
    out[i, d] = sum_p f[p, d] for every output partition i, so the result
    arrives already broadcast across partitions (no GPSIMD all-reduce).
    26 fp32r matmuls (free dim 512) accumulate into one PSUM bank.
    The same trick (ones/N stationary) turns the per-partition sum-of-squares
    column into a replicated scalar S2/N.
  * DVE: one fused tensor_tensor_reduce per feature tile produces the
    per-partition partial sum of squares (square + reduce in a single pass),
    plus the small x-path (x2) and the final combine.
  * ACT / GPSIMD: unused (their per-instruction overhead / boot cost is what
    made the previous version slow).

The host gather step sums the 8 partial outputs (the all-reduce of the
sharding hint).
"""

import sys

sys.path.insert(0, "/opt/trn_rl_repo")

import numpy as np

import concourse.bacc as bacc
import concourse.tile as tile
from concourse import mybir
from concourse import bass_utils

P = 128                    # SBUF partitions
B, D, N = 1024, 128, 100000
NCORES = 8
TPP = 100                  # feature rows per partition per core
RPC = P * TPP              # 12800 feature rows per core (padded shard)
PAD_N = RPC * NCORES       # 102400 rows after zero-padding
BT = B // P                # 8 x-rows per partition
# Feature DMA tiles (rows-per-partition each): 12 x 8 + 1 x 4 = 100.
# 8-row tiles are exactly two 512-wide fp32r matmuls; the small tile goes
# last so the post-stream dependency chain starts from the cheapest tile.
CHUNKS = [8] * 12 + [4]
MMF = 512                  # matmul moving free size (one PSUM bank of fp32)

F32 = mybir.dt.float32
F32R = mybir.dt.float32r
AX = mybir.AxisListType
OP = mybir.AluOpType


def _build():
    nc = bacc.Bacc("TRN2", debug=False, num_devices=NCORES)
    f_d = nc.dram_tensor("features", [RPC, D], F32, kind="ExternalInput").ap()
    x_d = nc.dram_tensor("x", [B, D], F32, kind="ExternalInput").ap()
    y_d = nc.dram_tensor("y", [P, BT], F32, kind="ExternalOutput").ap()

    # Row r of the shard maps to partition r // TPP, chunk r % TPP: each
    # partition reads one contiguous (TPP*D*4 B) run of DRAM per core.
    f_view = f_d.rearrange("(p t) d -> p t d", p=P)    # [128, 100, 128]
    x_view = x_d.rearrange("(p t) d -> p t d", p=P)    # [128, 8, 128]

    with tile.TileContext(nc) as tc:
        with (
            tc.tile_pool(name="fpool", bufs=1) as fpool,
            tc.tile_pool(name="scratch", bufs=1) as scratch,
            tc.tile_pool(name="small", bufs=1) as small,
            tc.tile_pool(name="psum", bufs=1, space="PSUM") as psum,
        ):
            # x first: its DVE work runs while features stream.
            xt = small.tile([P, BT, D], F32)
            nc.sync.dma_start(out=xt, in_=x_view)

            onesn = small.tile([P, P], F32)
            nc.vector.memset(onesn, 1.0 / N)
            # fp32r operands must come from an instruction that rounds to
            # fp32r; memset can't, so produce ones via a rounding copy.
            ones = small.tile([P, P], F32R)
            nc.vector.tensor_scalar_mul(ones, onesn, float(N))

            # x2 path on DVE (during the stream).
            xx = scratch.tile([P, BT, D], F32)
            nc.vector.tensor_mul(out=xx, in0=xt, in1=xt)
            x2cols = small.tile([P, BT], F32)
            nc.vector.tensor_reduce(out=x2cols, in_=xx, axis=AX.X, op=OP.add)
            x2s = small.tile([P, BT], F32)
            nc.vector.tensor_scalar_mul(x2s, x2cols, 1.0 / NCORES)

            # Feature stream: per tile one DMA, 1-2 accumulating PE matmuls
            # (S1), one fused DVE square+reduce (S2 partial).
            s1p = psum.tile([P, MMF], F32)
            nmm_total = sum(sz * D // MMF for sz in CHUNKS)
            sqacc = small.tile([P, len(CHUNKS)], F32)
            sqscr = [
                scratch.tile([P, CHUNKS[0] * D], F32, name=f"sqscr{k}")
                for k in range(2)
            ]
            mm_idx = 0
            off = 0
            for i, sz in enumerate(CHUNKS):
                ft = fpool.tile([P, sz, D], F32R, tag=f"ft{i}")
                nc.sync.dma_start(
                    out=ft, in_=f_view[:, off : off + sz, :].bitcast(F32R)
                )
                flat = ft.rearrange("p t d -> p (t d)")
                flat32 = flat.bitcast(F32)
                for j in range(sz * D // MMF):
                    nc.tensor.matmul(
                        s1p,
                        lhsT=ones,
                        rhs=flat[:, j * MMF : (j + 1) * MMF],
                        start=(mm_idx == 0),
                        stop=(mm_idx == nmm_total - 1),
                    )
                    mm_idx += 1
                # (tensor_tensor_reduce wedges the HW DVE ucode; use two ops)
                nc.vector.tensor_mul(
                    out=sqscr[i % 2][:, : sz * D], in0=flat32, in1=flat32
                )
                nc.vector.tensor_reduce(
                    out=sqacc[:, i : i + 1],
                    in_=sqscr[i % 2][:, : sz * D],
                    axis=AX.X,
                    op=OP.add,
                )
                off += sz

            # Tail: fold partials, broadcast S2 across partitions via PE,
            # then the combine.
            s2col = small.tile([P, 1], F32)
            nc.vector.tensor_reduce(
                out=s2col, in_=sqacc, axis=AX.X, op=OP.add
            )
            s2p = psum.tile([P, 1], F32)
            nc.tensor.matmul(s2p, lhsT=onesn, rhs=s2col, start=True, stop=True)

            # S1 fold: PSUM [128, 4*128] -> SBUF [128, 128] (replicated).
            s1f = small.tile([P, D], F32)
            nc.vector.tensor_reduce(
                out=s1f,
                in_=s1p.rearrange("p (t d) -> p d t", t=MMF // D),
                axis=AX.X,
                op=OP.add,
            )

            # dot_j[p] = x[p*8+j] . S1: multiply against S1 broadcast across
            # the 8 row-blocks via a stride-0 middle AP dim.
            import concourse.bass as bass
            s1rep = bass.AP(
                tensor=s1f.tensor, offset=s1f.offset,
                ap=[list(s1f.ap[0]), [0, BT], list(s1f.ap[1])],
            )
            xp = scratch.tile([P, BT, D], F32)
            nc.vector.tensor_mul(out=xp, in0=xt, in1=s1rep)
            dot8 = small.tile([P, BT], F32)
            nc.vector.tensor_reduce(out=dot8, in_=xp, axis=AX.X, op=OP.add)

            # y = x2/8 + (S2/N - (2/N)*dot); s2p already carries the 1/N.
            t1 = small.tile([P, BT], F32)
            nc.vector.tensor_scalar(
                out=t1, in0=dot8, scalar1=-2.0 / N, scalar2=s2p[:, 0:1],
                op0=OP.mult, op1=OP.add,
            )
            y_all = small.tile([P, BT], F32)
            nc.vector.tensor_add(y_all, t1, x2s)
            nc.sync.dma_start(out=y_d, in_=y_all)
    nc.compile()
    return nc


_nc_cache = None


def _get_nc():
    global _nc_cache
    if _nc_cache is None:
        _nc_cache = _build()
    return _nc_cache


def make_in_maps(x: np.ndarray, features: np.ndarray) -> list[dict[str, np.ndarray]]:
    x = np.ascontiguousarray(x, dtype=np.float32)
    features = np.ascontiguousarray(features, dtype=np.float32)
    padded = np.zeros((PAD_N, D), dtype=np.float32)
    padded[: features.shape[0]] = features
    return [
        {"features": padded[c * RPC : (c + 1) * RPC], "x": x}
        for c in range(NCORES)
    ]


def kernel(x: np.ndarray, features: np.ndarray, _trace: bool = False):
    nc = _get_nc()
    in_maps = make_in_maps(x, features)
    res = bass_utils.run_bass_kernel_spmd(
        nc, in_maps, core_ids=list(range(NCORES)), trace=_trace
    )
    out = np.zeros(B, dtype=np.float64)
    for c in range(NCORES):
        # y[p, t] holds output row p*BT + t, so row-major reshape is exact.
        out += res.results[c]["y"].reshape(B).astype(np.float64)
    out = out.astype(np.float32)
    if _trace:
        return out, res
    return out


# revision 10
# speedup vs baseline: 1.1780x; 1.1780x over previous
"""Distributed mean-squared-distance kernel for Trainium2 (8 NeuronCores).

Computes  out[b] = mean_n ||x[b] - features[n]||^2  for x:[1024,128],
features:[100000,128].

Because the mean is linear, the full [B, N] distance matrix is never needed:

    out[b] = ||x_b||^2 + (1/N) * sum_n ||f_n||^2 - (2/N) * x_b . (sum_n f_n)

Each core streams a 1/8 shard of `features` once (memory-bound roofline:
~6.55 MB/core at ~350 GB/s).  The shard is cast fp32->bf16 inside the DMA
(SWDGE inline cast): HBM traffic is unchanged, but every downstream engine
runs at its 16-bit fast path.  Precision loss is ~1e-5 relative -- the
noise-sensitive |x|^2 term stays fp32.

Engine split, all overlapped with the DMA stream:

  * PE: S1 = sum_n f_n via an all-ones bf16 stationary matrix -- the
    ones-matmul output is the column sum replicated across all 128 output
    partitions, so no cross-partition reduce is ever needed.  26 bf16
    matmuls (free dim 512) accumulate into one PSUM bank.  A final
    ones/N matmul likewise turns the per-partition sum-of-squares column
    into a replicated S2/N scalar.
  * DVE: per-tile square+reduce in bf16 (2 elem/cycle), the fp32 x-path,
    and the final combine.
  * ACT / GPSIMD-compute: unused (per-instruction overhead).

The host gather step sums the 8 partial outputs (the all-reduce of the
sharding hint).
"""

import sys

sys.path.insert(0, "/opt/trn_rl_repo")

import numpy as np

import concourse.bacc as bacc
import concourse.tile as tile
from concourse import mybir
from concourse import bass_utils

P = 128                    # SBUF partitions
B, D, N = 1024, 128, 100000
NCORES = 8
TPP = 100                  # feature rows per partition per core
RPC = P * TPP              # 12800 feature rows per core (padded shard)
PAD_N = RPC * NCORES       # 102400 rows after zero-padding
BT = B // P                # 8 x-rows per partition
# Feature DMA tiles (rows-per-partition each): 12 x 8 + 1 x 4 = 100.
# The small tile goes last so the post-stream dependency chain starts from
# the cheapest tile.
CHUNKS = [8] * 12 + [4]
MMF = 512                  # matmul moving free size (one PSUM bank of fp32)

F32 = mybir.dt.float32
BF16 = mybir.dt.bfloat16
AX = mybir.AxisListType
OP = mybir.AluOpType


def _build():
    nc = bacc.Bacc("TRN2", debug=False, num_devices=NCORES)
    f_d = nc.dram_tensor("features", [RPC, D], F32, kind="ExternalInput").ap()
    x_d = nc.dram_tensor("x", [B, D], F32, kind="ExternalInput").ap()
    y_d = nc.dram_tensor("y", [P, BT], F32, kind="ExternalOutput").ap()

    # Row r of the shard maps to partition r // TPP, chunk r % TPP: each
    # partition reads one contiguous (TPP*D*4 B) run of DRAM per core.
    f_view = f_d.rearrange("(p t) d -> p t d", p=P)    # [128, 100, 128]
    x_view = x_d.rearrange("(p t) d -> p t d", p=P)    # [128, 8, 128]

    with tile.TileContext(nc) as tc:
        with (
            tc.tile_pool(name="fpool", bufs=1) as fpool,
            tc.tile_pool(name="scratch", bufs=1) as scratch,
            tc.tile_pool(name="small", bufs=1) as small,
            tc.tile_pool(name="psum", bufs=1, space="PSUM") as psum,
        ):
            # x via HWDGE (no cast; the x-path stays fp32).
            xt = small.tile([P, BT, D], F32)
            nc.sync.dma_start(out=xt, in_=x_view)

            ones = small.tile([P, P], BF16)
            nc.vector.memset(ones, 1.0)
            onesn = small.tile([P, P], F32)
            nc.vector.memset(onesn, 1.0 / N)

            # x2 path on DVE (during the stream).
            xx = scratch.tile([P, BT, D], F32)
            nc.vector.tensor_mul(out=xx, in0=xt, in1=xt)
            x2cols = small.tile([P, BT], F32)
            nc.vector.tensor_reduce(out=x2cols, in_=xx, axis=AX.X, op=OP.add)
            x2s = small.tile([P, BT], F32)
            nc.vector.tensor_scalar_mul(x2s, x2cols, 1.0 / NCORES)

            # Feature stream: per tile one casting DMA, 1-2 accumulating PE
            # matmuls (S1), one DVE square + one DVE reduce (S2 partial).
            s1p = psum.tile([P, MMF], F32)
            nmm_total = sum(sz * D // MMF for sz in CHUNKS)
            sqacc = small.tile([P, len(CHUNKS)], F32)
            sqscr = [
                scratch.tile([P, CHUNKS[0] * D], BF16, name=f"sqscr{k}")
                for k in range(2)
            ]
            mm_idx = 0
            off = 0
            for i, sz in enumerate(CHUNKS):
                ft = fpool.tile([P, sz, D], BF16, tag=f"ft{i}")
                nc.gpsimd.dma_start(out=ft, in_=f_view[:, off : off + sz, :])
                flat = ft.rearrange("p t d -> p (t d)")
                for j in range(sz * D // MMF):
                    nc.tensor.matmul(
                        s1p,
                        lhsT=ones,
                        rhs=flat[:, j * MMF : (j + 1) * MMF],
                        start=(mm_idx == 0),
                        stop=(mm_idx == nmm_total - 1),
                    )
                    mm_idx += 1
                nc.vector.tensor_mul(
                    out=sqscr[i % 2][:, : sz * D], in0=flat, in1=flat
                )
                nc.vector.tensor_reduce(
                    out=sqacc[:, i : i + 1],
                    in_=sqscr[i % 2][:, : sz * D],
                    axis=AX.X,
                    op=OP.add,
                )
                off += sz

            # Tail: fold partials, broadcast S2 across partitions via PE,
            # then the combine.
            s2col = small.tile([P, 1], F32)
            nc.vector.tensor_reduce(
                out=s2col, in_=sqacc, axis=AX.X, op=OP.add
            )
            s2p = psum.tile([P, 1], F32)
            nc.tensor.matmul(s2p, lhsT=onesn, rhs=s2col, start=True, stop=True)

            # S1 fold: PSUM [128, 4*128] -> SBUF [128, 128] (replicated).
            s1f = small.tile([P, D], F32)
            nc.vector.tensor_reduce(
                out=s1f,
                in_=s1p.rearrange("p (t d) -> p d t", t=MMF // D),
                axis=AX.X,
                op=OP.add,
            )

            # dot_j[p] = x[p*8+j] . S1: one multiply against S1 broadcast
            # across the 8 row-blocks via a stride-0 middle AP dim.
            import concourse.bass as bass
            s1rep = bass.AP(
                tensor=s1f.tensor, offset=s1f.offset,
                ap=[list(s1f.ap[0]), [0, BT], list(s1f.ap[1])],
            )
            xp = scratch.tile([P, BT, D], F32)
            nc.vector.tensor_mul(out=xp, in0=xt, in1=s1rep)
            dot8 = small.tile([P, BT], F32)
            nc.vector.tensor_reduce(out=dot8, in_=xp, axis=AX.X, op=OP.add)

            # y = x2/8 + (S2/N - (2/N)*dot); s2p already carries the 1/N.
            t1 = small.tile([P, BT], F32)
            nc.vector.tensor_scalar(
                out=t1, in0=dot8, scalar1=-2.0 / N, scalar2=s2p[:, 0:1],
                op0=OP.mult, op1=OP.add,
            )
            y_all = small.tile([P, BT], F32)
            nc.vector.tensor_add(y_all, t1, x2s)
            nc.sync.dma_start(out=y_d, in_=y_all)
    nc.compile()
    return nc


_nc_cache = None


def _get_nc():
    global _nc_cache
    if _nc_cache is None:
        _nc_cache = _build()
    return _nc_cache


def make_in_maps(x: np.ndarray, features: np.ndarray) -> list[dict[str, np.ndarray]]:
    x = np.ascontiguousarray(x, dtype=np.float32)
    features = np.ascontiguousarray(features, dtype=np.float32)
    padded = np.zeros((PAD_N, D), dtype=np.float32)
    padded[: features.shape[0]] = features
    return [
        {"features": padded[c * RPC : (c + 1) * RPC], "x": x}
        for c in range(NCORES)
    ]


def kernel(x: np.ndarray, features: np.ndarray, _trace: bool = False):
    nc = _get_nc()
    in_maps = make_in_maps(x, features)
    res = bass_utils.run_bass_kernel_spmd(
        nc, in_maps, core_ids=list(range(NCORES)), trace=_trace
    )
    out = np.zeros(B, dtype=np.float64)
    for c in range(NCORES):
        # y[p, t] holds output row p*BT + t, so row-major reshape is exact.
        out += res.results[c]["y"].reshape(B).astype(np.float64)
    out = out.astype(np.float32)
    if _trace:
        return out, res
    return out


# revision 15
# speedup vs baseline: 1.3001x; 1.1036x over previous
"""Distributed mean-squared-distance kernel for Trainium2 (8 NeuronCores).

Computes  out[b] = mean_n ||x[b] - features[n]||^2  for x:[1024,128],
features:[100000,128].

Because the mean is linear, the full [B, N] distance matrix is never needed:

    out[b] = ||x_b||^2 + (1/N) * sum_n ||f_n||^2 - (2/N) * x_b . (sum_n f_n)

Each core streams a 1/8 shard of `features` once (memory-bound roofline:
~6.55 MB/core at ~350 GB/s).  The shard is cast fp32->bf16 inside the DMA
(SWDGE inline cast): HBM traffic is unchanged, but every downstream engine
runs at its 16-bit fast path.  Precision loss is ~1e-5 relative -- the
noise-sensitive |x|^2 term stays fp32.

Engine split, all overlapped with the DMA stream:

  * PE: S1 = sum_n f_n via an all-ones bf16 stationary matrix -- the
    ones-matmul output is the column sum replicated across all 128 output
    partitions, so no cross-partition reduce is ever needed.  26 bf16
    matmuls (free dim 512) accumulate into one PSUM bank.  A final
    ones/N matmul likewise turns the per-partition sum-of-squares column
    into a replicated S2/N scalar.
  * DVE: per-tile square+reduce in bf16 (2 elem/cycle), the fp32 x-path,
    and the final combine.
  * ACT / GPSIMD-compute: unused (per-instruction overhead).

The host gather step sums the 8 partial outputs (the all-reduce of the
sharding hint).
"""

import sys

sys.path.insert(0, "/opt/trn_rl_repo")

import numpy as np

import concourse.bacc as bacc
import concourse.tile as tile
from concourse import mybir
from concourse import bass_utils

P = 128                    # SBUF partitions
B, D, N = 1024, 128, 100000
NCORES = 8
TPP = 100                  # feature rows per partition per core
RPC = P * TPP              # 12800 feature rows per core (padded shard)
PAD_N = RPC * NCORES       # 102400 rows after zero-padding
BT = B // P                # 8 x-rows per partition
# Feature DMA tiles (rows-per-partition each): 12 x 8 + 1 x 4 = 100.
# The small tile goes last so the post-stream dependency chain starts from
# the cheapest tile.
CHUNKS = [8] * 12 + [4]
MMF = 512                  # matmul moving free size (one PSUM bank of fp32)

F32 = mybir.dt.float32
BF16 = mybir.dt.bfloat16
AX = mybir.AxisListType
OP = mybir.AluOpType


def _build():
    nc = bacc.Bacc("TRN2", debug=False, num_devices=NCORES)
    f_d = nc.dram_tensor("features", [RPC, D], F32, kind="ExternalInput").ap()
    x_d = nc.dram_tensor("x", [B, D], F32, kind="ExternalInput").ap()
    y_d = nc.dram_tensor("y", [P, BT], F32, kind="ExternalOutput").ap()

    # Row r of the shard maps to partition r // TPP, chunk r % TPP: each
    # partition reads one contiguous (TPP*D*4 B) run of DRAM per core.
    f_view = f_d.rearrange("(p t) d -> p t d", p=P)    # [128, 100, 128]
    x_view = x_d.rearrange("(p t) d -> p t d", p=P)    # [128, 8, 128]

    with tile.TileContext(nc) as tc:
        with (
            tc.tile_pool(name="fpool", bufs=1) as fpool,
            tc.tile_pool(name="scratch", bufs=1) as scratch,
            tc.tile_pool(name="small", bufs=1) as small,
            tc.tile_pool(name="psum", bufs=1, space="PSUM") as psum,
        ):
            # x via HWDGE (no cast; the x-path stays fp32).
            xt = small.tile([P, BT, D], F32)
            nc.sync.dma_start(out=xt, in_=x_view)

            ones = small.tile([P, P], BF16)
            nc.vector.memset(ones, 1.0)

            # x2 path on DVE (during the stream).
            xx = scratch.tile([P, BT, D], F32)
            nc.vector.tensor_mul(out=xx, in0=xt, in1=xt)
            x2cols = small.tile([P, BT], F32)
            nc.vector.tensor_reduce(out=x2cols, in_=xx, axis=AX.X, op=OP.add)
            x2s = small.tile([P, BT], F32)
            nc.vector.tensor_scalar_mul(x2s, x2cols, 1.0 / NCORES)

            # Feature stream: per tile one casting DMA, one DVE bf16 square,
            # and accumulating PE ones-matmuls for BOTH reductions -- S1 from
            # the raw tile, and the squared tile's column sums (toward S2)
            # into a second PSUM bank.  DVE never reduces the stream.
            s1p = psum.tile([P, MMF], F32)
            sqp = psum.tile([P, MMF], F32)
            nmm_total = sum(sz * D // MMF for sz in CHUNKS)
            sqscr = [
                scratch.tile([P, CHUNKS[0] * D], BF16, name=f"sqscr{k}")
                for k in range(3)
            ]
            mm_idx = 0
            off = 0
            for i, sz in enumerate(CHUNKS):
                ft = fpool.tile([P, sz, D], BF16, tag=f"ft{i}")
                nc.gpsimd.dma_start(out=ft, in_=f_view[:, off : off + sz, :])
                flat = ft.rearrange("p t d -> p (t d)")
                scr = sqscr[i % 3]
                nc.vector.tensor_mul(out=scr[:, : sz * D], in0=flat, in1=flat)
                for j in range(sz * D // MMF):
                    first = mm_idx == 0
                    last = mm_idx == nmm_total - 1
                    nc.tensor.matmul(
                        s1p,
                        lhsT=ones,
                        rhs=flat[:, j * MMF : (j + 1) * MMF],
                        start=first,
                        stop=last,
                        skip_group_check=True,
                    )
                    nc.tensor.matmul(
                        sqp,
                        lhsT=ones,
                        rhs=scr[:, j * MMF : (j + 1) * MMF],
                        start=first,
                        stop=last,
                        skip_group_check=True,
                    )
                    mm_idx += 1
                off += sz

            # Tail: fold the squared-column PSUM into the S2 scalar.  The
            # ones-matmul already summed over partitions (result replicated),
            # so the free-dim reduce leaves S2 on every partition directly.
            s2col = small.tile([P, 1], F32)
            nc.vector.tensor_reduce(out=s2col, in_=sqp, axis=AX.X, op=OP.add)
            s2n = small.tile([P, 1], F32)
            nc.vector.tensor_scalar_mul(s2n, s2col, 1.0 / N)

            # S1 fold: PSUM [128, 4*128] -> SBUF [128, 128] (replicated).
            s1f = small.tile([P, D], F32)
            nc.vector.tensor_reduce(
                out=s1f,
                in_=s1p.rearrange("p (t d) -> p d t", t=MMF // D),
                axis=AX.X,
                op=OP.add,
            )

            # dot_j[p] = x[p*8+j] . S1: one multiply against S1 broadcast
            # across the 8 row-blocks via a stride-0 middle AP dim.
            import concourse.bass as bass
            s1rep = bass.AP(
                tensor=s1f.tensor, offset=s1f.offset,
                ap=[list(s1f.ap[0]), [0, BT], list(s1f.ap[1])],
            )
            xp = scratch.tile([P, BT, D], F32)
            nc.vector.tensor_mul(out=xp, in0=xt, in1=s1rep)
            dot8 = small.tile([P, BT], F32)
            nc.vector.tensor_reduce(out=dot8, in_=xp, axis=AX.X, op=OP.add)

            # y = x2/8 + (S2/N - (2/N)*dot)
            t1 = small.tile([P, BT], F32)
            nc.vector.tensor_scalar(
                out=t1, in0=dot8, scalar1=-2.0 / N, scalar2=s2n[:, 0:1],
                op0=OP.mult, op1=OP.add,
            )
            y_all = small.tile([P, BT], F32)
            nc.vector.tensor_add(y_all, t1, x2s)
            nc.sync.dma_start(out=y_d, in_=y_all)
    nc.compile()
    return nc


_nc_cache = None


def _get_nc():
    global _nc_cache
    if _nc_cache is None:
        _nc_cache = _build()
    return _nc_cache


def make_in_maps(x: np.ndarray, features: np.ndarray) -> list[dict[str, np.ndarray]]:
    x = np.ascontiguousarray(x, dtype=np.float32)
    features = np.ascontiguousarray(features, dtype=np.float32)
    padded = np.zeros((PAD_N, D), dtype=np.float32)
    padded[: features.shape[0]] = features
    return [
        {"features": padded[c * RPC : (c + 1) * RPC], "x": x}
        for c in range(NCORES)
    ]


def kernel(x: np.ndarray, features: np.ndarray, _trace: bool = False):
    nc = _get_nc()
    in_maps = make_in_maps(x, features)
    res = bass_utils.run_bass_kernel_spmd(
        nc, in_maps, core_ids=list(range(NCORES)), trace=_trace
    )
    out = np.zeros(B, dtype=np.float64)
    for c in range(NCORES):
        # y[p, t] holds output row p*BT + t, so row-major reshape is exact.
        out += res.results[c]["y"].reshape(B).astype(np.float64)
    out = out.astype(np.float32)
    if _trace:
        return out, res
    return out


# revision 18
# speedup vs baseline: 1.4001x; 1.0769x over previous
"""Distributed mean-squared-distance kernel for Trainium2 (8 NeuronCores).

Computes  out[b] = mean_n ||x[b] - features[n]||^2  for x:[1024,128],
features:[100000,128].

Because the mean is linear, the full [B, N] distance matrix is never needed:

    out[b] = ||x_b||^2 + (1/N) * sum_n ||f_n||^2 - (2/N) * x_b . (sum_n f_n)

Each core streams a 1/8 shard of `features` once (memory-bound roofline:
~6.55 MB/core at ~350 GB/s).  The shard is cast fp32->bf16 inside the DMA
(SWDGE inline cast): HBM traffic is unchanged, but every downstream engine
runs at its 16-bit fast path.  Precision loss is ~1e-5 relative -- the
noise-sensitive |x|^2 term stays fp32.

Engine split, all overlapped with the DMA stream:

  * PE: S1 = sum_n f_n via an all-ones bf16 stationary matrix -- the
    ones-matmul output is the column sum replicated across all 128 output
    partitions, so no cross-partition reduce is ever needed.  26 bf16
    matmuls (free dim 512) accumulate into one PSUM bank.  A final
    ones/N matmul likewise turns the per-partition sum-of-squares column
    into a replicated S2/N scalar.
  * DVE: per-tile square+reduce in bf16 (2 elem/cycle), the fp32 x-path,
    and the final combine.
  * ACT / GPSIMD-compute: unused (per-instruction overhead).

The host gather step sums the 8 partial outputs (the all-reduce of the
sharding hint).
"""

import sys

sys.path.insert(0, "/opt/trn_rl_repo")

import numpy as np

import concourse.bacc as bacc
import concourse.tile as tile
from concourse import mybir
from concourse import bass_utils

P = 128                    # SBUF partitions
B, D, N = 1024, 128, 100000
NCORES = 8
TPP = 100                  # feature rows per partition per core
RPC = P * TPP              # 12800 feature rows per core (padded shard)
PAD_N = RPC * NCORES       # 102400 rows after zero-padding
BT = B // P                # 8 x-rows per partition
# Feature DMA tiles (rows-per-partition each): 12 x 8 + 1 x 4 = 100.
# The small tile goes last so the post-stream dependency chain starts from
# the cheapest tile.
CHUNKS = [8] * 12 + [4]
MMF = 512                  # matmul moving free size (one PSUM bank of fp32)

F32 = mybir.dt.float32
BF16 = mybir.dt.bfloat16
AX = mybir.AxisListType
OP = mybir.AluOpType


def _build():
    nc = bacc.Bacc("TRN2", debug=False, num_devices=NCORES)
    f_d = nc.dram_tensor("features", [RPC, D], F32, kind="ExternalInput").ap()
    x_d = nc.dram_tensor("x", [B, D], F32, kind="ExternalInput").ap()
    y_d = nc.dram_tensor("y", [P, BT], F32, kind="ExternalOutput").ap()

    # Row r of the shard maps to partition r // TPP, chunk r % TPP: each
    # partition reads one contiguous (TPP*D*4 B) run of DRAM per core.
    f_view = f_d.rearrange("(p t) d -> p t d", p=P)    # [128, 100, 128]
    x_view = x_d.rearrange("(p t) d -> p t d", p=P)    # [128, 8, 128]

    with tile.TileContext(nc) as tc:
        with (
            tc.tile_pool(name="fpool", bufs=1) as fpool,
            tc.tile_pool(name="scratch", bufs=1) as scratch,
            tc.tile_pool(name="small", bufs=1) as small,
            tc.tile_pool(name="psum", bufs=1, space="PSUM") as psum,
        ):
            # x via HWDGE (no cast; the x-path stays fp32).
            xt = small.tile([P, BT, D], F32)
            nc.sync.dma_start(out=xt, in_=x_view)

            ones = small.tile([P, P], BF16)
            nc.vector.memset(ones, 1.0)

            # The PE boots throttled (HAM clock gate, 1.2 GHz) and only
            # reaches 2.4 GHz after ~3.4us of sustained activity.  Keep it
            # busy with junk matmuls through the otherwise-dead window before
            # the first feature tile lands, so the real matmuls run warm.
            warmp = psum.tile([P, P], F32)
            for w in range(80):
                nc.tensor.matmul(
                    warmp, lhsT=ones, rhs=ones, start=True, stop=True,
                    skip_group_check=True,
                )

            # Feature stream: per tile one casting DMA, one DVE bf16 square,
            # and accumulating PE ones-matmuls for BOTH reductions -- S1 from
            # the raw tile, and the squared tile's column sums (toward S2)
            # into a second PSUM bank.  DVE never reduces the stream.
            s1p = psum.tile([P, MMF], F32)
            sqp = psum.tile([P, MMF], F32)
            nmm_total = sum(sz * D // MMF for sz in CHUNKS)
            sqscr = [
                scratch.tile([P, CHUNKS[0] * D], BF16, name=f"sqscr{k}")
                for k in range(3)
            ]
            mm_idx = 0
            off = 0
            for i, sz in enumerate(CHUNKS):
                ft = fpool.tile([P, sz, D], BF16, tag=f"ft{i}")
                nc.gpsimd.dma_start(out=ft, in_=f_view[:, off : off + sz, :])
                flat = ft.rearrange("p t d -> p (t d)")
                scr = sqscr[i % 3]
                nc.vector.tensor_mul(out=scr[:, : sz * D], in0=flat, in1=flat)
                for j in range(sz * D // MMF):
                    first = mm_idx == 0
                    last = mm_idx == nmm_total - 1
                    nc.tensor.matmul(
                        s1p,
                        lhsT=ones,
                        rhs=flat[:, j * MMF : (j + 1) * MMF],
                        start=first,
                        stop=last,
                        skip_group_check=True,
                    )
                    nc.tensor.matmul(
                        sqp,
                        lhsT=ones,
                        rhs=scr[:, j * MMF : (j + 1) * MMF],
                        start=first,
                        stop=last,
                        skip_group_check=True,
                    )
                    mm_idx += 1
                off += sz
                if i == 5:
                    # x2 path, emitted mid-stream: x has arrived by now, and
                    # putting it first would park unready instructions at the
                    # DVE queue head (wait-queue depth 4) and stall the tile
                    # squares behind them.
                    xx = scratch.tile([P, BT, D], F32)
                    nc.vector.tensor_mul(out=xx, in0=xt, in1=xt)
                    x2cols = small.tile([P, BT], F32)
                    nc.vector.tensor_reduce(
                        out=x2cols, in_=xx, axis=AX.X, op=OP.add
                    )
                    x2s = small.tile([P, BT], F32)
                    nc.vector.tensor_scalar_mul(x2s, x2cols, 1.0 / NCORES)

            # Tail.  S2 fold on the otherwise-idle ACT engine (in parallel
            # with the DVE fold chain): the ones-matmul already summed over
            # partitions (result replicated), so accumulating sqp's free dim
            # leaves S2/N on every partition directly.
            AF = mybir.ActivationFunctionType
            act_scr = scratch.tile([P, MMF], F32)
            s2n = small.tile([P, 1], F32)
            nc.scalar.activation(
                out=act_scr, in_=sqp, func=AF.Identity, scale=1.0 / N,
                accum_out=s2n,
            )

            # S1 fold: PSUM [128, 4*128] -> SBUF [128, 128] (replicated).
            s1f = small.tile([P, D], F32)
            nc.vector.tensor_reduce(
                out=s1f,
                in_=s1p.rearrange("p (t d) -> p d t", t=MMF // D),
                axis=AX.X,
                op=OP.add,
            )

            # dot_j[p] = x[p*8+j] . S1: one multiply against S1 broadcast
            # across the 8 row-blocks via a stride-0 middle AP dim.
            import concourse.bass as bass
            s1rep = bass.AP(
                tensor=s1f.tensor, offset=s1f.offset,
                ap=[list(s1f.ap[0]), [0, BT], list(s1f.ap[1])],
            )
            xp = scratch.tile([P, BT, D], F32)
            nc.vector.tensor_mul(out=xp, in0=xt, in1=s1rep)
            dot8 = small.tile([P, BT], F32)
            nc.vector.tensor_reduce(out=dot8, in_=xp, axis=AX.X, op=OP.add)

            # y = x2/8 + (S2/N - (2/N)*dot)
            t1 = small.tile([P, BT], F32)
            nc.vector.tensor_scalar(
                out=t1, in0=dot8, scalar1=-2.0 / N, scalar2=s2n[:, 0:1],
                op0=OP.mult, op1=OP.add,
            )
            y_all = small.tile([P, BT], F32)
            nc.vector.tensor_add(y_all, t1, x2s)
            nc.sync.dma_start(out=y_d, in_=y_all)
    nc.compile()
    return nc


_nc_cache = None


def _get_nc():
    global _nc_cache
    if _nc_cache is None:
        _nc_cache = _build()
    return _nc_cache


def make_in_maps(x: np.ndarray, features: np.ndarray) -> list[dict[str, np.ndarray]]:
    x = np.ascontiguousarray(x, dtype=np.float32)
    features = np.ascontiguousarray(features, dtype=np.float32)
    padded = np.zeros((PAD_N, D), dtype=np.float32)
    padded[: features.shape[0]] = features
    return [
        {"features": padded[c * RPC : (c + 1) * RPC], "x": x}
        for c in range(NCORES)
    ]


def kernel(x: np.ndarray, features: np.ndarray, _trace: bool = False):
    nc = _get_nc()
    in_maps = make_in_maps(x, features)
    res = bass_utils.run_bass_kernel_spmd(
        nc, in_maps, core_ids=list(range(NCORES)), trace=_trace
    )
    out = np.zeros(B, dtype=np.float64)
    for c in range(NCORES):
        # y[p, t] holds output row p*BT + t, so row-major reshape is exact.
        out += res.results[c]["y"].reshape(B).astype(np.float64)
    out = out.astype(np.float32)
    if _trace:
        return out, res
    return out
